# revision 1
# baseline (speedup 1.0000x reference)
"""Multi-head attention (B=8, N=1024, C=768, H=12) on 8 TRN2 NeuronCores.

Sharding: pure data-parallel over batch - core b computes attention for x[b].
Per-core Bass/Tile kernel, bf16 compute, f32 PSUM.

v2 schedule (no dup layout):
  qkv psum for pair hp lands with head a's 64 d-rows on partitions 0-63 and
  head b's on 64-127 (natural wqk column order). S matmuls pair
  (head a, rows 0-63) with (head b, rows 64-127) adjacent in the PE queue,
  so on HW they run concurrently via row-group tiling (tile_position
  auto-derived from base_partition). One [128,1024] PSUM->SBUF copy per
  role per pair; no mirror DMAs.

  E interleaved: e_ab[128, mt, nch, ab, 512] - one exp instruction per
  (mt, nch) covers both heads; one PSUM slot [128, (ab), 512] holds the
  S pair.

  O^T per head h: 16 MMs (8 subs x 2 nch), stationary v[sub,h] [128,65]
  (ones col fused -> sums in row 64), moving E. Pair hp's phase carries
  pair hp-1's O: head a' over groups 1-3, head b' over groups 5-7, leaving
  groups 0/4 as drain windows so the freed PSUM slot is back before the
  next O allocation.

  norm: sums row staged PSUM->SBUF on DVE (reciprocal_approx_fast misreads
  PSUM@partition-64 on real HW), DVE recip, Pool partition_broadcast; the
  multiply runs on Pool for group-3 heads and DVE for group-7 heads so
  Pool's phase-boundary queue stays short. The DVE drain copy frees the
  O PSUM slot ~2us after accumulation completes.

  PSUM budget: 4 two-bank slots - E pair-slot x2 (rotating, exp-paced),
  O accumulator x1, filler (qk chunk / v / proj partial) x1.

  tail: O(10), O(11) dense; proj partials (kt 0-3 prefilled as phase-5
  fillers) overlap the last norm chains; per-otp Act bias epilogue + DMA.
"""

import numpy as np
import ml_dtypes

B, N, C = 8, 1024, 768
H, D = 12, 64
SCALE = D ** -0.5
CT = C // 128        # 6 contraction tiles
NT = N // 128        # 8 token tiles
NCH = N // 512       # 2 n-chunks of 512
HP = H // 2          # 6 head pairs

_CACHE = {}


def _build_nc(loop_r=None):
    import concourse.bacc as bacc
    import concourse.mybir as mybir
    import concourse.tile as tile

    f32 = mybir.dt.float32
    bf16 = mybir.dt.bfloat16

    nc = bacc.Bacc("TRN2", target_bir_lowering=False, debug=False, num_devices=8)

    xT_d = nc.dram_tensor("xT", [C, N], bf16, kind="ExternalInput").ap()
    # wqk viewed [C, role(q|k), pair, 128] so one DMA descriptor per kt
    # carries q+k for a pair (or all five non-first pairs)
    wqk_d = nc.dram_tensor("wqk", [C, 2, CT, 128], bf16,
                           kind="ExternalInput").ap()
    wv_d = nc.dram_tensor("wv", [C, C], bf16, kind="ExternalInput").ap()
    wp_d = nc.dram_tensor("wp", [C, C], bf16, kind="ExternalInput").ap()
    pb_d = nc.dram_tensor("pb", [128, CT], f32, kind="ExternalInput").ap()
    out_d = nc.dram_tensor("out", [C, N], bf16, kind="ExternalOutput").ap()

    with tile.TileContext(nc) as tc:
        with (
            tc.tile_pool(name="const", bufs=1) as cpool,
            tc.tile_pool(name="E", bufs=2) as epool,
            tc.tile_pool(name="qk", bufs=4) as qkpool,
            tc.tile_pool(name="small", bufs=2) as spool,
            tc.tile_pool(name="y", bufs=4) as ypool,
            tc.tile_pool(name="ps", bufs=4, space="PSUM") as pspool,
        ):
            # ---- persistent SBUF tensors ----
            xT_sb = cpool.tile([128, CT, N], bf16)            # 12KB/part
            wqk_sb = cpool.tile([128, CT, 2, CT, 128], bf16)  # 18KB
            wv_sb = cpool.tile([128, CT, C], bf16)            # 9KB
            wp_sb = cpool.tile([128, CT, C], bf16)            # 9KB
            pb_sb = cpool.tile([128, CT], f32)
            # v and Onorm^T as per-nt / per-kt tiles: exact dependency
            # granularity (a write to one chunk never serializes readers
            # of the others)
            v_sb = [cpool.tile([128, H, D + 1], bf16, name=f"v{nt}")
                    for nt in range(NT)]                      # 12.2KB
            on_sb = [cpool.tile([128, NCH, 512], bf16, name=f"on{kt}")
                     for kt in range(CT)]                     # 12KB

            # input DMA, three queues by criticality: gpsimd's software
            # DGE (own issue budget) carries xT; the two HWDGE queues
            # (sync/scalar, shared issue bandwidth) carry pair-0 q+k and
            # wv first, then the remaining wqk pairs, then wp.
            # round-robin across the three DMA queues decides transfer
            # order: xT0 heads sync and wqk-pair0 heads scalar so the first
            # matmul's inputs both land in round one; xT1-5 stream on
            # gpsimd, then wv/wqk-rest/wp follow
            # xT0 split by n-half: the first matmul (nch0) needs only
            # columns 0:512, so it fires one transfer-round earlier
            nc.sync.dma_start(xT_sb[:, 0, 0:512], xT_d[0:128, 0:512])
            nc.sync.dma_start(xT_sb[:, 0, 512:1024], xT_d[0:128, 512:1024])
            for kt in range(CT):
                r = kt * 128
                if kt > 0:
                    nc.gpsimd.dma_start(xT_sb[:, kt, :], xT_d[r:r + 128, :])
                nc.scalar.dma_start(wqk_sb[:, kt, :, 0, :],
                                    wqk_d[r:r + 128, :, 0, :])
                nc.sync.dma_start(wv_sb[:, kt, :], wv_d[r:r + 128, :])
            for kt in range(CT):
                r = kt * 128
                nc.sync.dma_start(wqk_sb[:, kt, :, 1:CT, :],
                                  wqk_d[r:r + 128, :, 1:CT, :])
            for kt in range(CT):
                r = kt * 128
                nc.scalar.dma_start(wp_sb[:, kt, :], wp_d[r:r + 128, :])
            nc.gpsimd.dma_start(pb_sb[:], pb_d[:])
            # ones column fused into v (softmax sums emerge as O^T row 64)
            for nt in range(NT):
                nc.vector.memset(v_sb[nt][:, :, D:D + 1], 1.0)

            # HAM warm-up scratch: dummy matmuls fill the DMA-bound
            # startup idle so the clock gate is at full rate (and the
            # sim's p-state ramp warm) when the first real matmuls land
            scr = cpool.tile([128, 512], bf16, name="scr")
            nc.vector.memset(scr[:], 1.0)

            def body():
                qks = {}      # (hp, role) -> SBUF tile; ("ps",hp,role) -> psum
                vps = {}
                ps_w = pspool.tile([128, 512], f32, tag="ps", name="warm")
                for _ in range(12):
                    nc.tensor.matmul(ps_w[:], scr[:, 0:128], scr[:],
                                     start=True, stop=True)

                def emit_qk_chunk(hp, role, lo, hi):
                    key = ("ps", hp, role)
                    if lo == 0:
                        qks[key] = pspool.tile([128, NCH, 512], f32, tag="ps",
                                               name=f"qk{hp}r{role}")
                    ps = qks[key]
                    for kt in range(lo, hi):
                        for nch in range(NCH):
                            nc.tensor.matmul(
                                ps[:, nch, :],
                                wqk_sb[:, kt, role, hp, :],
                                xT_sb[:, kt, nch * 512:(nch + 1) * 512],
                                start=(kt == 0), stop=(kt == CT - 1),
                            )
                    if hi == CT:
                        t = qkpool.tile([128, NCH, 512], bf16, tag="qk",
                                        name=f"qk{hp}r{role}sb")
                        if hp == 0:
                            # startup only: per-nch copies shorten the
                            # DMA->qk->S(0) critical path by half a copy
                            for nch in range(NCH):
                                nc.vector.tensor_copy(t[:, nch, :],
                                                      ps[:, nch, :])
                        else:
                            nc.vector.tensor_copy(t[:], ps[:, :, :])
                        qks[(hp, role)] = t
                        del qks[key]

                def emit_v_chunk(nt, lo, hi):
                    if lo == 0:
                        vps[nt] = pspool.tile([128, 2, 8, 64], f32, tag="ps",
                                              name=f"v{nt}")
                    ps = vps[nt]
                    for kt in range(lo, hi):
                        for och in range(2):
                            nc.tensor.matmul(
                                ps[:, och, 0:6, :],
                                xT_sb[:, kt, nt * 128:(nt + 1) * 128],
                                wv_sb[:, kt, och * 384:(och + 1) * 384],
                                start=(kt == 0), stop=(kt == CT - 1),
                            )
                    if hi == CT:
                        for och in range(2):
                            nc.vector.tensor_copy(
                                v_sb[nt][:, och * 6:(och + 1) * 6, 0:D],
                                ps[:, och, 0:6, :],
                            )
                        del vps[nt]

                fillers = []

                def take_fillers(k):
                    for _ in range(min(k, len(fillers))):
                        fillers.pop(0)()

                def queue_qk_chunks(hp):
                    for role in (0, 1):
                        for kt in range(CT):
                            fillers.append(
                                lambda hp=hp, role=role, kt=kt:
                                emit_qk_chunk(hp, role, kt, kt + 1))

                def queue_v_chunks(nt):
                    fillers.append(lambda: emit_v_chunk(nt, 0, 3))
                    fillers.append(lambda: emit_v_chunk(nt, 3, CT))

                def norm(h, ps_o, tail=False):
                    """O^T[0:64]/sums(row64) -> on_sb. The DVE drain copy
                    frees the PSUM slot early; Pool does broadcast+mult."""
                    hp, odd = h // 2, h % 2
                    # reciprocal_approx_fast misreads PSUM@partition-64 on
                    # HW - stage the sums row through SBUF first
                    sm = spool.tile([1, NCH, 512], f32, tag="sm")
                    rec = spool.tile([1, NCH, 512], f32, tag="rec")
                    ou = spool.tile([64, NCH, 512], bf16, tag="ou")
                    R = spool.tile([64, NCH, 512], f32, tag="R")
                    if tail:
                        # stage-major emission: all recips precede all
                        # mults on DVE, so the in-order DVE queue never
                        # blocks at a mult waiting for Pool's broadcast
                        for nch in range(NCH):
                            nc.scalar.copy(sm[:, nch, :],
                                           ps_o[64:65, nch, :])
                            nc.vector.reciprocal_approx_fast(
                                rec[:, nch, :], sm[:, nch, :])
                            nc.gpsimd.partition_broadcast(R[:, nch, :],
                                                          rec[:, nch, :])
                        for nch in range(NCH):
                            nc.scalar.copy(ou[:, nch, :],
                                           ps_o[0:64, nch, :])
                        for nch in range(NCH):
                            nc.vector.tensor_tensor(
                                on_sb[hp][64 * odd:64 * odd + 64, nch, :],
                                ou[:, nch, :], R[:, nch, :],
                                op=mybir.AluOpType.mult,
                            )
                        return
                    # sm and ou (the two PSUM readers) run back-to-back on
                    # DVE so the O slot frees before the recip, which only
                    # reads sm's SBUF copy
                    nc.vector.tensor_copy(sm[:], ps_o[64:65, :, :])
                    nc.vector.tensor_copy(ou[:], ps_o[0:64, :, :])
                    nc.vector.reciprocal_approx_fast(rec[:], sm[:])
                    nc.gpsimd.partition_broadcast(R[:], rec[:])
                    # odd heads norm at group 7: their multiply goes to
                    # DVE so Pool's phase-boundary queue stays short
                    eng = nc.vector if odd else nc.gpsimd
                    eng.tensor_tensor(
                        on_sb[hp][64 * odd:64 * odd + 64, :, :],
                        ou[:], R[:], op=mybir.AluOpType.mult,
                    )

                # ---- startup: eager qk(pair 0) pipelined with input DMA.
                # q+k land together per kt (merged descriptor): interleave
                # roles so each kt landing feeds 4 MMs, not 2 ----
                for kt in range(CT):
                    emit_qk_chunk(0, 0, kt, kt + 1)
                    emit_qk_chunk(0, 1, kt, kt + 1)
                emit_v_chunk(0, 0, CT)
                emit_v_chunk(1, 0, CT)

                E_prev = None
                pj = {}
                stg = {}

                def stage_proj(otp):
                    t = ypool.tile([128, NCH, 512], bf16, tag="stg",
                                   name=f"stg{otp}")
                    nc.vector.tensor_copy(t[:], pj[otp][:, :, :])
                    stg[otp] = t
                    del pj[otp]

                def proj_chunk(otp, kts, lo=0, stop_kt=CT - 1,
                               nch_major=False):
                    if kts[0] == lo:
                        pj[otp] = pspool.tile([128, NCH, 512], f32, tag="ps",
                                              name=f"pj{otp}k{lo}")
                    # nch_major: nch0's accumulation closes 6 MMs early so
                    # its Act/DMA epilogue overlaps nch1's matmuls
                    order = ([(kt, nch) for nch in range(NCH) for kt in kts]
                             if nch_major else
                             [(kt, nch) for kt in kts for nch in range(NCH)])
                    for kt, nch in order:
                        nc.tensor.matmul(
                            pj[otp][:, nch, :],
                            wp_sb[:, kt, otp * 128:(otp + 1) * 128],
                            on_sb[kt][:, nch, :],
                            start=(kt == lo), stop=(kt == stop_kt),
                        )

                def proj_fin(otp, split_engines=False):
                    # per-nch epilogue so the Act copy and output DMA of
                    # nch0 overlap the tail's remaining PE work.
                    # split_engines (last otps): nch1's bias-add runs on
                    # idle DVE so the two fins proceed in parallel instead
                    # of serializing on Act's queue
                    yt = ypool.tile([128, NCH, 512], bf16, tag="yt")
                    for nch in range(NCH):
                        if split_engines and nch == 1:
                            nc.vector.tensor_scalar_add(
                                yt[:, nch, :], pj[otp][:, nch, :],
                                pb_sb[:, otp:otp + 1])
                        else:
                            nc.scalar.activation(
                                yt[:, nch, :], pj[otp][:, nch, :],
                                mybir.ActivationFunctionType.Identity,
                                bias=pb_sb[:, otp:otp + 1],
                            )
                        # split across software/hardware DGE queues
                        eng = nc.gpsimd if nch == 0 else nc.sync
                        eng.dma_start(
                            out_d[otp * 128:(otp + 1) * 128,
                                  nch * 512:(nch + 1) * 512],
                            yt[:, nch, :])

                for hp in range(HP):
                    q_sb, k_sb = qks[(hp, 0)], qks[(hp, 1)]
                    E_ab = epool.tile([128, NT, NCH, 2, 512], bf16, tag="E",
                                      name=f"E{hp}")
                    if hp == 0:
                        # qk(1) first: its SBUF copy gates phase 1's S MMs
                        queue_qk_chunks(1)
                        for nt in range(2, 6):
                            queue_v_chunks(nt)
                    elif hp == 1:
                        # v(6/7) first: O(pair 0) group 3 reads them mid-phase
                        for nt in range(6, 8):
                            queue_v_chunks(nt)
                        queue_qk_chunks(2)
                    elif hp < HP - 1:
                        queue_qk_chunks(hp + 1)
                    else:
                        # phase 5: proj otp-0/1/2 partials over ready kt 0-3,
                        # staged to SBUF via DVE (transient PSUM slot each)
                        for otp in (0, 1):
                            for kt in range(3):
                                fillers.append(
                                    lambda otp=otp, kt=kt:
                                    proj_chunk(otp, [kt], stop_kt=3))
                            fillers.append(
                                lambda otp=otp:
                                proj_chunk(otp, [3], stop_kt=3))
                            fillers.append(
                                lambda otp=otp: stage_proj(otp))

                    # O-prev sub-tiles per group: groups 1-3 head a',
                    # groups 5-7 head b'; groups 0/4 are PSUM drain windows.
                    osub = {1: (0, 3), 2: (3, 6), 3: (6, 8),
                            5: (0, 3), 6: (3, 6), 7: (6, 8)}
                    ps_o = None
                    for mt in range(NT):
                        c0, c1 = mt // 4, (mt % 4) * 128
                        for nch in range(NCH):
                            ps_s = pspool.tile([128, 2, 512], f32, tag="ps",
                                               name=f"s{hp}m{mt}n{nch}")
                            nc.tensor.matmul(
                                ps_s[:, 0, :],
                                k_sb[0:64, c0, c1:c1 + 128],
                                q_sb[0:64, nch, :], start=True, stop=True,
                            )
                            nc.tensor.matmul(
                                ps_s[:, 1, :],
                                k_sb[64:128, c0, c1:c1 + 128],
                                q_sb[64:128, nch, :], start=True, stop=True,
                            )
                            nc.scalar.activation(
                                E_ab[:, mt, nch, :, :], ps_s[:, :, :],
                                mybir.ActivationFunctionType.Exp, scale=SCALE,
                            )
                        if E_prev is not None and mt in osub:
                            h_prev = 2 * (hp - 1) + (0 if mt <= 3 else 1)
                            ab = h_prev % 2
                            lo, hi = osub[mt]
                            if lo == 0:
                                ps_o = pspool.tile([65, NCH, 512], f32,
                                                   tag="ps", name=f"o{h_prev}")
                            for sub in range(lo, hi):
                                for nch in range(NCH):
                                    nc.tensor.matmul(
                                        ps_o[:, nch, :],
                                        v_sb[sub][:, h_prev, :],
                                        E_prev[:, sub, nch, ab, :],
                                        start=(sub == 0), stop=(sub == NT - 1),
                                    )
                            if hi == NT:
                                norm(h_prev, ps_o)
                        # front-load pops so next pair's qk copy lands
                        # mid-phase, but keep late groups fed
                        nfill = 3 if (mt < 2 or len(fillers) > 16) else 2
                        take_fillers(nfill)
                    take_fillers(len(fillers))
                    E_prev = E_ab

                # ---- tail: O(pair 5) + projections, dense on PE ----
                ha, hb = 2 * (HP - 1), 2 * (HP - 1) + 1

                def emit_o_dense(h):
                    ps = pspool.tile([65, NCH, 512], f32, tag="ps",
                                     name=f"ot{h}")
                    ab = h % 2
                    for sub in range(NT):
                        for nch in range(NCH):
                            nc.tensor.matmul(
                                ps[:, nch, :], v_sb[sub][:, h, :],
                                E_prev[:, sub, nch, ab, :],
                                start=(sub == 0), stop=(sub == NT - 1),
                            )
                    return ps

                def proj_fin_staged(otp):
                    # (kt4+kt5 psum + bias) + staged kt0-3 partial, on DVE,
                    # per-nch so each half's output DMA issues early
                    yt = ypool.tile([128, NCH, 512], bf16, tag="yt")
                    for nch in range(NCH):
                        nc.vector.scalar_tensor_tensor(
                            yt[:, nch, :], pj[otp][:, nch, :],
                            pb_sb[:, otp:otp + 1], stg[otp][:, nch, :],
                            op0=mybir.AluOpType.add, op1=mybir.AluOpType.add,
                        )
                        eng = nc.gpsimd if nch == 0 else nc.sync
                        eng.dma_start(
                            out_d[otp * 128:(otp + 1) * 128,
                                  nch * 512:(nch + 1) * 512],
                            yt[:, nch, :])

                ps_oa = emit_o_dense(ha)
                norm(ha, ps_oa, tail=True)
                ps_ob = emit_o_dense(hb)
                # overlap norm(11) chain with proj partial blocks
                proj_chunk(2, range(0, 5))
                norm(hb, ps_ob, tail=True)
                proj_chunk(3, range(0, 5))
                proj_chunk(0, [4, 5], lo=4)
                proj_fin_staged(0)
                proj_chunk(1, [4, 5], lo=4)
                proj_fin_staged(1)
                proj_chunk(2, [5])
                proj_fin(2)
                proj_chunk(3, [5])
                proj_fin(3)
                for otp in range(4, CT):
                    proj_chunk(otp, range(0, CT), nch_major=True)
                    proj_fin(otp, split_engines=True)

            if loop_r is not None:
                with tc.For_i(0, loop_r):
                    body()
            else:
                body()

    nc.compile()
    return nc


def _get_nc():
    if "nc" not in _CACHE:
        _CACHE["nc"] = _build_nc()
    return _CACHE["nc"]


def kernel(x, qkv_w, proj_w, proj_b):
    from concourse.bass_utils import run_bass_kernel_spmd

    nc = _get_nc()
    bf = ml_dtypes.bfloat16
    wqk = np.ascontiguousarray(qkv_w[:2 * C].T).astype(bf).reshape(C, 2, CT, 128)
    wv = np.ascontiguousarray(qkv_w[2 * C:].T).astype(bf)
    wp = np.ascontiguousarray(proj_w.T).astype(bf)
    pb = np.ascontiguousarray(proj_b.reshape(CT, 128).T).astype(np.float32)
    in_maps = []
    for i in range(B):
        in_maps.append({
            "xT": np.ascontiguousarray(x[i].T).astype(bf),
            "wqk": wqk, "wv": wv, "wp": wp, "pb": pb,
        })
    res = run_bass_kernel_spmd(nc, in_maps, core_ids=list(range(B)))
    out = np.stack([res.results[i]["out"].astype(np.float32).T for i in range(B)])
    return np.ascontiguousarray(out)



# revision 6
# speedup vs baseline: 1.0216x; 1.0216x over previous
"""Multi-head attention (B=8, N=1024, C=768, H=12) on 8 TRN2 NeuronCores.

Sharding: pure data-parallel over batch - core b computes attention for x[b].
Per-core Bass/Tile kernel, bf16 compute, f32 PSUM.

v3 schedule (orientation-B O):
  qkv/S unchanged from v2: qkv psum pairs heads on partition halves; S
  matmuls per (mt, nch) write [128 keys, 2 heads, 512 q] PSUM; one exp per
  (mt, nch) covers both heads -> E_ab[128, mt, nch, ab, 512] bf16.

  O restructured: out[q, d] = E_chunk^T @ v with E as the (free) stationary
  operand and v [128, 65] moving (ones col -> softmax sums in col 64).
  Per (head, qtile): 8 accumulating matmuls of 65 columns instead of the
  old [65, 512] orientation - halves the O column count on the PE.

  norm: DVE reciprocal of the PSUM sums column + per-partition
  tensor_scalar_mul -> normalized ob[q, d] bf16 in SBUF.

  transpose: proj needs on[d, q]; ob^T comes from a PE matmul against an
  identity matrix (ldweights are free): ot[64, 128] = ob_qt^T @ I, then one
  DVE copy per half moves [64, 4, 128] PSUM -> on_sb.

  PSUM budget (16KB/partition): tag "s" 2x[128,2,512]f32 (S double-buffer,
  also startup qk pair-0), tag "o" 2x[128,4,128]f32 (O accumulator halves
  and transpose outputs alternate through the same two slots), tag "f"
  1x4KB (warmup, qk/v/proj fillers, serial by construction).

  Per phase hp (S/exp of pair hp, O of pair hp-1), chains spread over mts:
    mt0 a-lo qt01 | mt1 a-lo qt23 +norm | mt2 b-lo qt01 | mt3 b-lo qt23
    +norm, T(a-lo)+copy | mt4 T(b-lo)+copy, a-hi qt45 | mt5 a-hi qt67
    +norm | mt6 b-hi qt45 | mt7 b-hi qt67 +norm, T(a-hi)+copy, T(b-hi)+copy
  so each "o" slot's next allocation waits only on work finished ~2 mts ago.

  tail: O(pair 5) same pattern dense; proj kt0-3 partials prestaged as
  phase-4/5 fillers (DVE stage to SBUF), tail does kt4-5 + fused epilogue
  (partial + psum + bias on DVE) + split-queue output DMA.
"""

import numpy as np
import ml_dtypes

B, N, C = 8, 1024, 768
H, D = 12, 64
SCALE = D ** -0.5
CT = C // 128        # 6 contraction tiles
NT = N // 128        # 8 token tiles
NCH = N // 512       # 2 n-chunks of 512
HP = H // 2          # 6 head pairs

_CACHE = {}


def _build_nc(loop_r=None):
    import concourse.bacc as bacc
    import concourse.mybir as mybir
    import concourse.tile as tile

    f32 = mybir.dt.float32
    bf16 = mybir.dt.bfloat16

    nc = bacc.Bacc("TRN2", target_bir_lowering=False, debug=False, num_devices=8)

    xT_d = nc.dram_tensor("xT", [C, N], bf16, kind="ExternalInput").ap()
    wqk_d = nc.dram_tensor("wqk", [C, 2, CT, 128], bf16,
                           kind="ExternalInput").ap()
    wv_d = nc.dram_tensor("wv", [C, C], bf16, kind="ExternalInput").ap()
    wp_d = nc.dram_tensor("wp", [C, C], bf16, kind="ExternalInput").ap()
    pb_d = nc.dram_tensor("pb", [128, CT], f32, kind="ExternalInput").ap()
    eye_d = nc.dram_tensor("eye", [128, 128], bf16, kind="ExternalInput").ap()
    out_d = nc.dram_tensor("out", [C, N], bf16, kind="ExternalOutput").ap()

    with tile.TileContext(nc) as tc:
        with (
            tc.tile_pool(name="const", bufs=1) as cpool,
            tc.tile_pool(name="E", bufs=2) as epool,
            tc.tile_pool(name="qk", bufs=4) as qkpool,
            tc.tile_pool(name="small", bufs=4) as spool,
            tc.tile_pool(name="y", bufs=4) as ypool,
            tc.tile_pool(name="ps", bufs=1, space="PSUM") as pspool,
        ):
            # ---- persistent SBUF tensors ----
            xT_sb = cpool.tile([128, CT, N], bf16)            # 12KB/part
            wqk_sb = cpool.tile([128, CT, 2, CT, 128], bf16)  # 18KB
            wv_sb = cpool.tile([128, CT, C], bf16)            # 9KB
            wp_sb = cpool.tile([128, CT, C], bf16)            # 9KB
            pb_sb = cpool.tile([128, CT], f32)
            eye_sb = cpool.tile([128, 128], bf16)
            v_sb = [cpool.tile([128, H, D + 1], bf16, name=f"v{nt}")
                    for nt in range(NT)]                      # 12.2KB
            on_sb = [cpool.tile([128, NCH, 512], bf16, name=f"on{kt}")
                     for kt in range(CT)]                     # 12KB
            stg_sb = [cpool.tile([128, NCH, 512], bf16, name=f"stg{otp}")
                      for otp in range(CT)]                   # 12KB

            # warmup scratch memset first so PE ramp starts ASAP
            scr = cpool.tile([128, 256], bf16, name="scr")
            nc.vector.memset(scr[:], 1.0)

            # input DMA, three queues by criticality (see v2 notes)
            nc.sync.dma_start(xT_sb[:, 0, 0:512], xT_d[0:128, 0:512])
            nc.sync.dma_start(xT_sb[:, 0, 512:1024], xT_d[0:128, 512:1024])
            for kt in range(CT):
                r = kt * 128
                if kt > 0:
                    nc.gpsimd.dma_start(xT_sb[:, kt, :], xT_d[r:r + 128, :])
                nc.scalar.dma_start(wqk_sb[:, kt, :, 0, :],
                                    wqk_d[r:r + 128, :, 0, :])
                nc.sync.dma_start(wv_sb[:, kt, :], wv_d[r:r + 128, :])
            for kt in range(CT):
                r = kt * 128
                nc.sync.dma_start(wqk_sb[:, kt, :, 1:CT, :],
                                  wqk_d[r:r + 128, :, 1:CT, :])
            for kt in range(CT):
                r = kt * 128
                nc.scalar.dma_start(wp_sb[:, kt, :], wp_d[r:r + 128, :])
            nc.gpsimd.dma_start(pb_sb[:], pb_d[:])
            nc.gpsimd.dma_start(eye_sb[:], eye_d[:])
            # ones column fused into v (softmax sums emerge as O col 64)
            for nt in range(NT):
                nc.vector.memset(v_sb[nt][:, :, D:D + 1], 1.0)

            def body():
                qks = {}      # (hp, role) -> SBUF tile; ("ps",hp,role) -> psum
                vps = {}
                ps_w = pspool.tile([128, 2, 512], f32, tag="f", name="warm")
                for i in range(12):
                    nc.tensor.matmul(ps_w[:, 0, 0:256], scr[:, 0:128],
                                     scr[:], start=True, stop=True)

                def emit_qk_chunk(hp, role, lo, hi, tag="f"):
                    key = ("ps", hp, role)
                    if lo == 0:
                        qks[key] = pspool.tile([128, NCH, 512], f32, tag=tag,
                                               bufs=(2 if tag == "s" else 1),
                                               name=f"qk{hp}r{role}")
                    ps = qks[key]
                    for kt in range(lo, hi):
                        for nch in range(NCH):
                            nc.tensor.matmul(
                                ps[:, nch, :],
                                wqk_sb[:, kt, role, hp, :],
                                xT_sb[:, kt, nch * 512:(nch + 1) * 512],
                                start=(kt == 0), stop=(kt == CT - 1),
                            )
                    if hi == CT:
                        t = qkpool.tile([128, NCH, 512], bf16, tag="qk",
                                        name=f"qk{hp}r{role}sb")
                        if hp == 0:
                            for nch in range(NCH):
                                nc.vector.tensor_copy(t[:, nch, :],
                                                      ps[:, nch, :])
                        else:
                            nc.vector.tensor_copy(t[:], ps[:, :, :])
                        qks[(hp, role)] = t
                        del qks[key]

                def emit_v_chunk(nt, lo, hi):
                    if lo == 0:
                        vps[nt] = pspool.tile([128, 2, 8, 64], f32, tag="f",
                                              name=f"v{nt}")
                    ps = vps[nt]
                    for kt in range(lo, hi):
                        for och in range(2):
                            nc.tensor.matmul(
                                ps[:, och, 0:6, :],
                                xT_sb[:, kt, nt * 128:(nt + 1) * 128],
                                wv_sb[:, kt, och * 384:(och + 1) * 384],
                                start=(kt == 0), stop=(kt == CT - 1),
                            )
                    if hi == CT:
                        for och in range(2):
                            nc.vector.tensor_copy(
                                v_sb[nt][:, och * 6:(och + 1) * 6, 0:D],
                                ps[:, och, 0:6, :],
                            )
                        del vps[nt]

                fillers = []

                def take_fillers(k):
                    for _ in range(min(k, len(fillers))):
                        fillers.pop(0)()

                def queue_qk_chunks(hp):
                    for role in (0, 1):
                        for kt in range(CT):
                            fillers.append(
                                lambda hp=hp, role=role, kt=kt:
                                emit_qk_chunk(hp, role, kt, kt + 1))

                def queue_v_chunks(nt):
                    fillers.append(lambda: emit_v_chunk(nt, 0, 3))
                    fillers.append(lambda: emit_v_chunk(nt, 3, CT))

                pj = {}

                def stage_proj(otp):
                    nc.vector.tensor_copy(stg_sb[otp][:], pj[otp][:, :, :])
                    del pj[otp]

                def proj_chunk(otp, kts, lo, stop_kt, nchs=(0, 1), tag="f"):
                    if kts[0] == lo:
                        pj[otp] = pspool.tile([128, NCH, 512], f32, tag=tag,
                                              bufs=(2 if tag == "s" else 1),
                                              name=f"pj{otp}k{lo}")
                    for kt in kts:
                        for nch in nchs:
                            nc.tensor.matmul(
                                pj[otp][:, nch, :],
                                wp_sb[:, kt, otp * 128:(otp + 1) * 128],
                                on_sb[kt][:, nch, :],
                                start=(kt == lo), stop=(kt == stop_kt),
                            )

                def queue_proj_prefill(otps, last):
                    # prefill kt 0..last (on[last] must be ready a phase
                    # before the pops land), stage partial to SBUF
                    for otp in otps:
                        for kt in range(last + 1):
                            fillers.append(
                                lambda otp=otp, kt=kt:
                                proj_chunk(otp, [kt], 0, last))
                        fillers.append(lambda otp=otp: stage_proj(otp))

                # ---- startup: eager qk(pair 0) on the S psum slots ----
                for kt in range(CT):
                    emit_qk_chunk(0, 0, kt, kt + 1, tag="s")
                    emit_qk_chunk(0, 1, kt, kt + 1, tag="s")
                emit_v_chunk(0, 0, CT)
                emit_v_chunk(1, 0, CT)

                E_prev = None

                # ---- O machinery (orientation B) ----
                def o_chain(ps_o, h, half, qi):
                    """ps_o[:, qi, 0:65] += E^T @ v over all 8 key subs."""
                    nch = half
                    qc = qi
                    ab = h % 2
                    for sub in range(NT):
                        nc.tensor.matmul(
                            ps_o[:, qi, 0:D + 1],
                            E_prev[:, sub, nch, ab, qc * 128:(qc + 1) * 128],
                            v_sb[sub][:, h, :],
                            start=(sub == 0), stop=(sub == NT - 1),
                        )

                def o_norm(ps_o, ob, rec):
                    """rec = 1/sums, ob[q, qt, d] = O * rec (bf16)."""
                    nc.vector.reciprocal_approx_fast(rec[:, :], ps_o[:, :, D])
                    for qi in range(4):
                        nc.vector.tensor_scalar_mul(
                            ob[:, qi, :], ps_o[:, qi, 0:D],
                            rec[:, qi:qi + 1])

                def o_transpose(ob, h, half):
                    """ot = ob^T per qtile via eye matmul; copy to on_sb."""
                    hp0, odd = h // 2, h % 2
                    ot = pspool.tile([64, 4, 128], f32, tag="o",
                                     bufs=2, name=f"ot{h}h{half}")
                    for qi in range(4):
                        nc.tensor.matmul(ot[0:64, qi, :], ob[:, qi, :],
                                         eye_sb[:], start=True, stop=True)
                    nc.vector.tensor_copy(
                        on_sb[hp0][64 * odd:64 * odd + 64, half, :],
                        ot[0:64, :, :])

                def new_o(h, half):
                    ps_o = pspool.tile([128, 4, 128], f32, tag="o",
                                       bufs=2, name=f"o{h}h{half}")
                    ob = ypool.tile([128, 4, D], bf16, tag="ob", bufs=4,
                                    name=f"ob{h}h{half}")
                    rec = spool.tile([128, 4], f32, tag="rec",
                                     name=f"rec{h}h{half}")
                    return ps_o, ob, rec

                for hp in range(HP):
                    q_sb, k_sb = qks[(hp, 0)], qks[(hp, 1)]
                    E_ab = epool.tile([128, NT, NCH, 2, 512], bf16, tag="E",
                                      name=f"E{hp}")
                    if hp == 0:
                        queue_qk_chunks(1)
                        for nt in range(2, 8):
                            queue_v_chunks(nt)
                    elif hp < HP - 1:
                        queue_qk_chunks(hp + 1)
                        if hp == HP - 2:
                            queue_proj_prefill((0, 1), 2)
                    else:
                        queue_proj_prefill((2, 3, 4, 5), 3)

                    ha, hb = (2 * (hp - 1), 2 * (hp - 1) + 1)
                    st = {}
                    for mt in range(NT):
                        for nch in range(NCH):
                            ps_s = pspool.tile([128, 2, 512], f32, tag="s",
                                               bufs=2, name=f"s{hp}m{mt}n{nch}")
                            c0, c1 = mt // 4, (mt % 4) * 128
                            nc.tensor.matmul(
                                ps_s[:, 0, :],
                                k_sb[0:64, c0, c1:c1 + 128],
                                q_sb[0:64, nch, :], start=True, stop=True,
                            )
                            nc.tensor.matmul(
                                ps_s[:, 1, :],
                                k_sb[64:128, c0, c1:c1 + 128],
                                q_sb[64:128, nch, :], start=True, stop=True,
                            )
                            nc.scalar.activation(
                                E_ab[:, mt, nch, :, :], ps_s[:, :, :],
                                mybir.ActivationFunctionType.Exp, scale=SCALE,
                            )
                        if E_prev is not None:
                            if mt == 0:
                                st["alo"] = new_o(ha, 0)
                                o_chain(st["alo"][0], ha, 0, 0)
                                o_chain(st["alo"][0], ha, 0, 1)
                            elif mt == 1:
                                o_chain(st["alo"][0], ha, 0, 2)
                                o_chain(st["alo"][0], ha, 0, 3)
                                o_norm(*st["alo"])
                            elif mt == 2:
                                st["blo"] = new_o(hb, 0)
                                o_chain(st["blo"][0], hb, 0, 0)
                                o_chain(st["blo"][0], hb, 0, 1)
                            elif mt == 3:
                                o_chain(st["blo"][0], hb, 0, 2)
                                o_chain(st["blo"][0], hb, 0, 3)
                                o_norm(*st["blo"])
                                o_transpose(st["alo"][1], ha, 0)
                            elif mt == 4:
                                o_transpose(st["blo"][1], hb, 0)
                                st["ahi"] = new_o(ha, 1)
                                o_chain(st["ahi"][0], ha, 1, 0)
                                o_chain(st["ahi"][0], ha, 1, 1)
                            elif mt == 5:
                                o_chain(st["ahi"][0], ha, 1, 2)
                                o_chain(st["ahi"][0], ha, 1, 3)
                                o_norm(*st["ahi"])
                            elif mt == 6:
                                st["bhi"] = new_o(hb, 1)
                                o_chain(st["bhi"][0], hb, 1, 0)
                                o_chain(st["bhi"][0], hb, 1, 1)
                            elif mt == 7:
                                o_chain(st["bhi"][0], hb, 1, 2)
                                o_chain(st["bhi"][0], hb, 1, 3)
                                o_norm(*st["bhi"])
                        nfill = 3 if (mt < 2 or len(fillers) > 16) else 2
                        take_fillers(nfill)
                    if E_prev is not None:
                        o_transpose(st["ahi"][1], ha, 1)
                        take_fillers(2)
                        o_transpose(st["bhi"][1], hb, 1)
                    take_fillers(len(fillers))
                    E_prev = E_ab

                # ---- tail: O(pair 5) + proj kt4-5 + epilogues ----
                ha, hb = 2 * (HP - 1), 2 * (HP - 1) + 1

                def proj_fin_staged(otp):
                    yt = ypool.tile([128, NCH, 512], bf16, tag="yt")
                    for nch in range(NCH):
                        nc.vector.scalar_tensor_tensor(
                            yt[:, nch, :], pj[otp][:, nch, :],
                            pb_sb[:, otp:otp + 1], stg_sb[otp][:, nch, :],
                            op0=mybir.AluOpType.add, op1=mybir.AluOpType.add,
                        )
                        eng = nc.gpsimd if nch == 0 else nc.sync
                        eng.dma_start(
                            out_d[otp * 128:(otp + 1) * 128,
                                  nch * 512:(nch + 1) * 512],
                            yt[:, nch, :])

                # proj psums: otp0/1 -> "s" slots, otp2 -> "f"; otp3-5 reuse
                # the same slots after the fins free them. otp0/1 start at
                # kt3 (prefill staged kt0-2), otp2-5 at kt4 (staged kt0-3).
                proj_chunk(0, [3], 3, 5, tag="s")   # pads the ot-copy wait
                alo = new_o(ha, 0)
                o_chain(alo[0], ha, 0, 0)
                o_chain(alo[0], ha, 0, 1)
                o_chain(alo[0], ha, 0, 2)
                o_chain(alo[0], ha, 0, 3)
                o_norm(*alo)
                blo = new_o(hb, 0)
                o_chain(blo[0], hb, 0, 0)
                o_chain(blo[0], hb, 0, 1)
                # proj kt4 partials overlap the remaining O work (on4 ready)
                proj_chunk(1, [3], 3, 5, tag="s")
                o_chain(blo[0], hb, 0, 2)
                o_chain(blo[0], hb, 0, 3)
                o_norm(*blo)
                o_transpose(alo[1], ha, 0)
                proj_chunk(0, [4], 3, 5)
                o_transpose(blo[1], hb, 0)
                proj_chunk(1, [4], 3, 5)
                ahi = new_o(ha, 1)
                o_chain(ahi[0], ha, 1, 0)
                o_chain(ahi[0], ha, 1, 1)
                proj_chunk(2, [4], 4, 5, tag="f")
                # on5 nch0 complete: kt5 nch0 for otp0-2
                proj_chunk(0, [5], 3, 5, nchs=(0,))
                o_chain(ahi[0], ha, 1, 2)
                o_chain(ahi[0], ha, 1, 3)
                o_norm(*ahi)
                proj_chunk(1, [5], 3, 5, nchs=(0,))
                bhi = new_o(hb, 1)
                o_chain(bhi[0], hb, 1, 0)
                o_chain(bhi[0], hb, 1, 1)
                proj_chunk(2, [5], 4, 5, nchs=(0,))
                o_chain(bhi[0], hb, 1, 2)
                o_chain(bhi[0], hb, 1, 3)
                o_norm(*bhi)
                o_transpose(ahi[1], ha, 1)
                o_transpose(bhi[1], hb, 1)
                # nch1 tails + remaining otps
                proj_chunk(0, [5], 3, 5, nchs=(1,))
                proj_fin_staged(0)
                proj_chunk(1, [5], 3, 5, nchs=(1,))
                proj_fin_staged(1)
                proj_chunk(2, [5], 4, 5, nchs=(1,))
                proj_fin_staged(2)
                for otp, tag in ((3, "s"), (4, "s"), (5, "f")):
                    proj_chunk(otp, [4, 5], 4, 5, tag=tag)
                    proj_fin_staged(otp)

            if loop_r is not None:
                with tc.For_i(0, loop_r):
                    body()
            else:
                body()

    nc.compile()
    return nc


def _get_nc():
    if "nc" not in _CACHE:
        _CACHE["nc"] = _build_nc()
    return _CACHE["nc"]


def kernel(x, qkv_w, proj_w, proj_b):
    from concourse.bass_utils import run_bass_kernel_spmd

    nc = _get_nc()
    bf = ml_dtypes.bfloat16
    wqk = np.ascontiguousarray(qkv_w[:2 * C].T).astype(bf).reshape(C, 2, CT, 128)
    wv = np.ascontiguousarray(qkv_w[2 * C:].T).astype(bf)
    wp = np.ascontiguousarray(proj_w.T).astype(bf)
    pb = np.ascontiguousarray(proj_b.reshape(CT, 128).T).astype(np.float32)
    eye = np.eye(128, dtype=bf)
    in_maps = []
    for i in range(B):
        in_maps.append({
            "xT": np.ascontiguousarray(x[i].T).astype(bf),
            "wqk": wqk, "wv": wv, "wp": wp, "pb": pb, "eye": eye,
        })
    res = run_bass_kernel_spmd(nc, in_maps, core_ids=list(range(B)))
    out = np.stack([res.results[i]["out"].astype(np.float32).T for i in range(B)])
    return np.ascontiguousarray(out)


# revision 23
# speedup vs baseline: 1.0450x; 1.0229x over previous
"""Multi-head attention (B=8, N=1024, C=768, H=12) on 8 TRN2 NeuronCores.

Sharding: pure data-parallel over batch - core b computes attention for x[b].
Per-core Bass/Tile kernel, bf16 compute, f32 PSUM.

v3 schedule (orientation-B O):
  qkv/S unchanged from v2: qkv psum pairs heads on partition halves; S
  matmuls per (mt, nch) write [128 keys, 2 heads, 512 q] PSUM; one exp per
  (mt, nch) covers both heads -> E_ab[128, mt, nch, ab, 512] bf16.

  O restructured: out[q, d] = E_chunk^T @ v with E as the (free) stationary
  operand and v [128, 65] moving (ones col -> softmax sums in col 64).
  Per (head, qtile): 8 accumulating matmuls of 65 columns instead of the
  old [65, 512] orientation - halves the O column count on the PE.

  norm: DVE reciprocal of the PSUM sums column + per-partition
  tensor_scalar_mul -> normalized ob[q, d] bf16 in SBUF.

  transpose: proj needs on[d, q]; ob^T comes from a PE matmul against an
  identity matrix (ldweights are free): ot[64, 128] = ob_qt^T @ I, then one
  DVE copy per half moves [64, 4, 128] PSUM -> on_sb.

  PSUM budget (16KB/partition): tag "s" 2x[128,2,512]f32 (S double-buffer,
  also startup qk pair-0), tag "o" 2x[128,4,128]f32 (O accumulator halves
  and transpose outputs alternate through the same two slots), tag "f"
  1x4KB (warmup, qk/v/proj fillers, serial by construction).

  Per phase hp (S/exp of pair hp, O of pair hp-1), chains spread over mts:
    mt0 a-lo qt01 | mt1 a-lo qt23 +norm | mt2 b-lo qt01 | mt3 b-lo qt23
    +norm, T(a-lo)+copy | mt4 T(b-lo)+copy, a-hi qt45 | mt5 a-hi qt67
    +norm | mt6 b-hi qt45 | mt7 b-hi qt67 +norm, T(a-hi)+copy, T(b-hi)+copy
  so each "o" slot's next allocation waits only on work finished ~2 mts ago.

  tail: O(pair 5) same pattern dense; proj kt0-3 partials prestaged as
  phase-4/5 fillers (DVE stage to SBUF), tail does kt4-5 + fused epilogue
  (partial + psum + bias on DVE) + split-queue output DMA.
"""

import numpy as np
import ml_dtypes

B, N, C = 8, 1024, 768
H, D = 12, 64
SCALE = D ** -0.5
CT = C // 128        # 6 contraction tiles
NT = N // 128        # 8 token tiles
NCH = N // 512       # 2 n-chunks of 512
HP = H // 2          # 6 head pairs

_CACHE = {}


def _build_nc(loop_r=None):
    import concourse.bacc as bacc
    import concourse.mybir as mybir
    import concourse.tile as tile

    f32 = mybir.dt.float32
    bf16 = mybir.dt.bfloat16

    nc = bacc.Bacc("TRN2", target_bir_lowering=False, debug=False, num_devices=8)

    xT_d = nc.dram_tensor("xT", [C, N], bf16, kind="ExternalInput").ap()
    # weights declared row-tiled [CT, 128, ...] so one multi-descriptor DMA
    # (single HWDGE issue) can stage a whole weight with dst partitions =
    # the inner 128 rows
    # wqk layout [role, pair, row128, kt, d]: per-(role, pair-slice) DMA
    # opts to 3 dims (row, pair, kt*d) on both sides
    wqk_d = nc.dram_tensor("wqk", [2, CT, 128, CT, 128], bf16,
                           kind="ExternalInput").ap()
    wv_d = nc.dram_tensor("wv", [CT, 128, C], bf16, kind="ExternalInput").ap()
    wp_d = nc.dram_tensor("wp", [CT, 128, C], bf16, kind="ExternalInput").ap()
    pb_d = nc.dram_tensor("pb", [128, CT], f32, kind="ExternalInput").ap()
    eye_d = nc.dram_tensor("eye", [128, 128], bf16, kind="ExternalInput").ap()
    out_d = nc.dram_tensor("out", [C, N], bf16, kind="ExternalOutput").ap()

    with tile.TileContext(nc) as tc:
        with (
            tc.tile_pool(name="const", bufs=1) as cpool,
            tc.tile_pool(name="E", bufs=2) as epool,
            tc.tile_pool(name="qk", bufs=4) as qkpool,
            tc.tile_pool(name="small", bufs=4) as spool,
            tc.tile_pool(name="y", bufs=4) as ypool,
            tc.tile_pool(name="ps", bufs=1, space="PSUM") as pspool,
        ):
            # ---- persistent SBUF tensors ----
            xT_sb = cpool.tile([128, CT, N], bf16)            # 12KB/part
            wqk_sb = cpool.tile([128, 2, CT, C], bf16)        # 18KB
            wv_sb = cpool.tile([128, CT, C], bf16)            # 9KB
            wp_sb = cpool.tile([128, CT, C], bf16)            # 9KB
            pb_sb = cpool.tile([128, CT], f32)
            eye_sb = cpool.tile([128, 128], bf16)
            v_sb = [cpool.tile([128, H, D + 1], bf16, name=f"v{nt}")
                    for nt in range(NT)]                      # 12.2KB
            on_sb = [cpool.tile([128, NCH, 512], bf16, name=f"on{kt}")
                     for kt in range(CT)]                     # 12KB
            stg_sb = [cpool.tile([128, NCH, 512], bf16, name=f"stg{otp}")
                      for otp in range(CT)]                   # 12KB

            # warmup scratch memset first so PE ramp starts ASAP
            scr = cpool.tile([128, 256], bf16, name="scr")
            nc.vector.memset(scr[:], 1.0)

            # input DMA: everything HWDGE goes on the SYNC queue (SP has no
            # compute - issuing from scalar/vector blocks that engine's SEQ
            # behind the shared HWDGE device). Ordered by first use:
            # qk0 inputs, then pair-1 weights, wv, pairs 2-5, wp. xT rows
            # 1/3 ride the gpsimd SWDGE (separate device) in parallel.
            nc.sync.dma_start(xT_sb[:, 0, 0:512], xT_d[0:128, 0:512])
            nc.sync.dma_start(xT_sb[:, 0, 512:1024], xT_d[0:128, 512:1024])
            for role in range(2):
                nc.sync.dma_start(
                    wqk_sb[:, role, 0, :],
                    wqk_d[role, 0, :, :, :].rearrange("b c d -> b (c d)"))
            nc.gpsimd.dma_start(xT_sb[:, 1, :], xT_d[128:256, :])
            nc.gpsimd.dma_start(xT_sb[:, 3, :], xT_d[384:512, :])
            nc.sync.dma_start(xT_sb[:, 2, :], xT_d[256:384, :])
            nc.sync.dma_start(xT_sb[:, 4, :], xT_d[512:640, :])
            nc.sync.dma_start(xT_sb[:, 5, :], xT_d[640:768, :])
            for role in range(2):
                nc.sync.dma_start(
                    wqk_sb[:, role, 1, :],
                    wqk_d[role, 1, :, :, :].rearrange("b c d -> b (c d)"))
            nc.sync.dma_start(wv_sb[:, :, :],
                              wv_d.rearrange("a b c -> b a c"))
            for role in range(2):
                nc.sync.dma_start(
                    wqk_sb[:, role, 2:CT, :],
                    wqk_d[role, 2:CT, :, :, :]
                    .rearrange("a b c d -> b a (c d)"))
            nc.sync.dma_start(wp_sb[:, :, :],
                              wp_d.rearrange("a b c -> b a c"))
            nc.gpsimd.dma_start(pb_sb[:], pb_d[:])
            nc.gpsimd.dma_start(eye_sb[:], eye_d[:])
            # ones column fused into v (softmax sums emerge as O col 64)
            for nt in range(NT):
                nc.vector.memset(v_sb[nt][:, :, D:D + 1], 1.0)

            def body():
                qks = {}      # (hp, role) -> SBUF tile; ("ps",hp,role) -> psum
                vps = {}
                ps_w = pspool.tile([128, 2, 512], f32, tag="f", name="warm")
                for i in range(12):
                    nc.tensor.matmul(ps_w[:, 0, 0:256], scr[:, 0:128],
                                     scr[:], start=True, stop=True)

                def emit_qk_chunk(hp, role, lo, hi, tag="f",
                                  defer_copy=False):
                    key = ("ps", hp, role)
                    if lo == 0:
                        qks[key] = pspool.tile([128, NCH, 512], f32, tag=tag,
                                               bufs=(2 if tag == "s" else 1),
                                               name=f"qk{hp}r{role}")
                    ps = qks[key]
                    for kt in range(lo, hi):
                        for nch in range(NCH):
                            nc.tensor.matmul(
                                ps[:, nch, :],
                                wqk_sb[:, role, hp, kt * 128:kt * 128 + 128],
                                xT_sb[:, kt, nch * 512:(nch + 1) * 512],
                                start=(kt == 0), stop=(kt == CT - 1),
                            )
                    if hi == CT and not defer_copy:
                        qk_copy(hp, role)

                def qk_copy(hp, role):
                    # PSUM->SBUF copy emitted at a fixed low-DVE-pressure
                    # slot so it never delays a chain-gating norm/otcopy
                    key = ("ps", hp, role)
                    ps = qks[key]
                    t = qkpool.tile([128, NCH, 512], bf16, tag="qk",
                                    name=f"qk{hp}r{role}sb")
                    if hp == 0:
                        # startup: split copies across Act (idle) and DVE
                        for nch in range(NCH):
                            if role == 0:
                                nc.scalar.copy(t[:, nch, :], ps[:, nch, :])
                            else:
                                nc.vector.tensor_copy(t[:, nch, :],
                                                      ps[:, nch, :])
                    else:
                        nc.vector.tensor_copy(t[:], ps[:, :, :])
                    qks[(hp, role)] = t
                    del qks[key]

                def emit_v_chunk(nt, lo, hi):
                    if lo == 0:
                        vps[nt] = pspool.tile([128, 2, 8, 64], f32, tag="f",
                                              name=f"v{nt}")
                    ps = vps[nt]
                    for kt in range(lo, hi):
                        for och in range(2):
                            nc.tensor.matmul(
                                ps[:, och, 0:6, :],
                                xT_sb[:, kt, nt * 128:(nt + 1) * 128],
                                wv_sb[:, kt, och * 384:(och + 1) * 384],
                                start=(kt == 0), stop=(kt == CT - 1),
                            )
                    if hi == CT:
                        for och in range(2):
                            nc.vector.tensor_copy(
                                v_sb[nt][:, och * 6:(och + 1) * 6, 0:D],
                                ps[:, och, 0:6, :],
                            )
                        del vps[nt]

                fillers = []

                def take_fillers(k):
                    for _ in range(min(k, len(fillers))):
                        fillers.pop(0)()

                def queue_qk_chunks(hp):
                    for role in (0, 1):
                        for kt in range(CT):
                            fillers.append(
                                lambda hp=hp, role=role, kt=kt:
                                emit_qk_chunk(hp, role, kt, kt + 1,
                                              defer_copy=(kt == CT - 1)))

                def queue_v_chunks(nt):
                    fillers.append(lambda: emit_v_chunk(nt, 0, 3))
                    fillers.append(lambda: emit_v_chunk(nt, 3, CT))

                pj = {}

                def stage_proj(otp):
                    nc.vector.tensor_copy(stg_sb[otp][:], pj[otp][:, :, :])
                    del pj[otp]

                def proj_chunk(otp, kts, lo, stop_kt, nchs=(0, 1), tag="f"):
                    if kts[0] == lo:
                        pj[otp] = pspool.tile([128, NCH, 512], f32, tag=tag,
                                              bufs=(2 if tag == "s" else 1),
                                              name=f"pj{otp}k{lo}")
                    for kt in kts:
                        for nch in nchs:
                            nc.tensor.matmul(
                                pj[otp][:, nch, :],
                                wp_sb[:, kt, otp * 128:(otp + 1) * 128],
                                on_sb[kt][:, nch, :],
                                start=(kt == lo), stop=(kt == stop_kt),
                            )

                def queue_proj_prefill(otps, last):
                    # prefill kt 0..last (on[last] must be ready a phase
                    # before the pops land), stage partial to SBUF
                    for otp in otps:
                        for kt in range(last + 1):
                            fillers.append(
                                lambda otp=otp, kt=kt:
                                proj_chunk(otp, [kt], 0, last))
                        fillers.append(lambda otp=otp: stage_proj(otp))

                # ---- startup: eager qk(pair 0) on the S psum slots;
                # v0/v1 go through the filler queue (wv lands late).
                # A warm matmul between chunks fills each DMA-wait gap so
                # the PE p-state ramp never resets ----
                for kt in range(CT):
                    emit_qk_chunk(0, 0, kt, kt + 1, tag="s")
                    emit_qk_chunk(0, 1, kt, kt + 1, tag="s")
                    nc.tensor.matmul(ps_w[:, 1, 0:256], scr[:, 0:128],
                                     scr[:], start=True, stop=True)
                # pair-1 role-0 fills the copy window before phase 0
                for kt in range(CT):
                    emit_qk_chunk(1, 0, kt, kt + 1,
                                  defer_copy=(kt == CT - 1))

                E_prev = None

                # ---- O machinery (orientation B) ----
                def o_chain(ps_o, h, half, qi):
                    """ps_o[:, qi, 0:65] += E^T @ v over all 8 key subs."""
                    nch = half
                    qc = qi
                    ab = h % 2
                    for sub in range(NT):
                        nc.tensor.matmul(
                            ps_o[:, qi, 0:D + 1],
                            E_prev[:, sub, nch, ab, qc * 128:(qc + 1) * 128],
                            v_sb[sub][:, h, :],
                            start=(sub == 0), stop=(sub == NT - 1),
                        )

                def o_norm(ps_o, ob, rec, act=False):
                    """rec = 1/sums, ob[q, qt, d] = O * rec (bf16). act=True
                    runs the multiplies on the Activation engine (tail)."""
                    nc.vector.reciprocal_approx_fast(rec[:, :], ps_o[:, :, D])
                    for qi in range(4):
                        if act:
                            nc.scalar.activation(
                                ob[:, qi, :], ps_o[:, qi, 0:D],
                                mybir.ActivationFunctionType.Identity,
                                scale=rec[:, qi:qi + 1])
                        else:
                            nc.vector.tensor_scalar_mul(
                                ob[:, qi, :], ps_o[:, qi, 0:D],
                                rec[:, qi:qi + 1])

                def o_transpose(ob, h, half, act=False):
                    """ot = ob^T per qtile via eye matmul; copy to on_sb."""
                    hp0, odd = h // 2, h % 2
                    ot = pspool.tile([64, 4, 128], f32, tag="o",
                                     bufs=2, name=f"ot{h}h{half}")
                    for qi in range(4):
                        nc.tensor.matmul(ot[0:64, qi, :], ob[:, qi, :],
                                         eye_sb[:], start=True, stop=True)
                    dst = on_sb[hp0][64 * odd:64 * odd + 64, half, :]
                    if act:
                        nc.scalar.copy(dst, ot[0:64, :, :])
                    else:
                        nc.vector.tensor_copy(dst, ot[0:64, :, :])

                def new_o(h, half):
                    ps_o = pspool.tile([128, 4, 128], f32, tag="o",
                                       bufs=2, name=f"o{h}h{half}")
                    ob = ypool.tile([128, 4, D], bf16, tag="ob", bufs=4,
                                    name=f"ob{h}h{half}")
                    rec = spool.tile([128, 4], f32, tag="rec",
                                     name=f"rec{h}h{half}")
                    return ps_o, ob, rec

                for hp in range(HP):
                    q_sb, k_sb = qks[(hp, 0)], qks[(hp, 1)]
                    E_ab = epool.tile([128, NT, NCH, 2, 512], bf16, tag="E",
                                      name=f"E{hp}")
                    if hp == 0:
                        for kt in range(CT):
                            fillers.append(
                                lambda kt=kt:
                                emit_qk_chunk(1, 1, kt, kt + 1,
                                              defer_copy=(kt == CT - 1)))
                        for nt in range(NT):
                            queue_v_chunks(nt)
                    elif hp < HP - 1:
                        queue_qk_chunks(hp + 1)
                        if hp == HP - 2:
                            queue_proj_prefill((0, 1), 2)
                    else:
                        queue_proj_prefill((2, 3, 4, 5), 3)

                    ha, hb = (2 * (hp - 1), 2 * (hp - 1) + 1)
                    st = {}
                    for mt in range(NT):
                        for nch in range(NCH):
                            ps_s = pspool.tile([128, 2, 512], f32, tag="s",
                                               bufs=2, name=f"s{hp}m{mt}n{nch}")
                            c0, c1 = mt // 4, (mt % 4) * 128
                            nc.tensor.matmul(
                                ps_s[:, 0, :],
                                k_sb[0:64, c0, c1:c1 + 128],
                                q_sb[0:64, nch, :], start=True, stop=True,
                            )
                            nc.tensor.matmul(
                                ps_s[:, 1, :],
                                k_sb[64:128, c0, c1:c1 + 128],
                                q_sb[64:128, nch, :], start=True, stop=True,
                            )
                            nc.scalar.activation(
                                E_ab[:, mt, nch, :, :], ps_s[:, :, :],
                                mybir.ActivationFunctionType.Exp, scale=SCALE,
                            )
                        if mt == 2 and hp < HP - 1:
                            qk_copy(hp + 1, 0)
                        if mt == 6 and hp < HP - 1:
                            qk_copy(hp + 1, 1)
                        if E_prev is not None:
                            if mt == 0:
                                st["alo"] = new_o(ha, 0)
                                o_chain(st["alo"][0], ha, 0, 0)
                                o_chain(st["alo"][0], ha, 0, 1)
                            elif mt == 1:
                                o_chain(st["alo"][0], ha, 0, 2)
                                o_chain(st["alo"][0], ha, 0, 3)
                                o_norm(*st["alo"])
                            elif mt == 2:
                                st["blo"] = new_o(hb, 0)
                                o_chain(st["blo"][0], hb, 0, 0)
                                o_chain(st["blo"][0], hb, 0, 1)
                            elif mt == 3:
                                o_chain(st["blo"][0], hb, 0, 2)
                                o_chain(st["blo"][0], hb, 0, 3)
                                o_norm(*st["blo"])
                                o_transpose(st["alo"][1], ha, 0)
                            elif mt == 4:
                                o_transpose(st["blo"][1], hb, 0)
                                st["ahi"] = new_o(ha, 1)
                                o_chain(st["ahi"][0], ha, 1, 0)
                                o_chain(st["ahi"][0], ha, 1, 1)
                            elif mt == 5:
                                o_chain(st["ahi"][0], ha, 1, 2)
                                o_chain(st["ahi"][0], ha, 1, 3)
                                o_norm(*st["ahi"])
                            elif mt == 6:
                                st["bhi"] = new_o(hb, 1)
                                o_chain(st["bhi"][0], hb, 1, 0)
                                o_chain(st["bhi"][0], hb, 1, 1)
                            elif mt == 7:
                                o_chain(st["bhi"][0], hb, 1, 2)
                                o_chain(st["bhi"][0], hb, 1, 3)
                                o_norm(*st["bhi"])
                        nfill = 3 if (mt < 2 or len(fillers) > 16) else 2
                        take_fillers(nfill)
                    if E_prev is not None:
                        o_transpose(st["ahi"][1], ha, 1)
                        take_fillers(2)
                        o_transpose(st["bhi"][1], hb, 1)
                    take_fillers(len(fillers))
                    E_prev = E_ab

                # ---- tail: O(pair 5) + proj kt4-5 + epilogues ----
                ha, hb = 2 * (HP - 1), 2 * (HP - 1) + 1

                yts = {}

                def proj_fin_nch(otp, nch, ap=None):
                    # Act drains the psum + bias (idle at tail), DVE adds the
                    # staged kt0-partial at 2x bf16 rate, split-queue DMA out
                    if otp not in yts:
                        yts[otp] = (
                            ypool.tile([128, NCH, 512], bf16, tag="y1",
                                       name=f"y1_{otp}"),
                            ypool.tile([128, NCH, 512], bf16, tag="yt",
                                       name=f"yt_{otp}"),
                        )
                    y1, yt = yts[otp]
                    nc.scalar.activation(
                        y1[:, nch, :],
                        pj[otp][:, nch, :] if ap is None else ap,
                        mybir.ActivationFunctionType.Identity,
                        bias=pb_sb[:, otp:otp + 1],
                    )
                    nc.vector.tensor_tensor(
                        yt[:, nch, :], y1[:, nch, :], stg_sb[otp][:, nch, :],
                        op=mybir.AluOpType.add,
                    )
                    eng = nc.gpsimd if nch == 0 else nc.sync
                    eng.dma_start(
                        out_d[otp * 128:(otp + 1) * 128,
                              nch * 512:(nch + 1) * 512],
                        yt[:, nch, :])

                def proj_fin_staged(otp):
                    for nch in range(NCH):
                        proj_fin_nch(otp, nch)

                def proj_tail(otp, nch, tag):
                    # wave-2 proj: kt4-5 straight into a freed small slot,
                    # fin immediately (Act + DVE + DMA)
                    pjn = pspool.tile([128, 512], f32, tag=tag,
                                      bufs=(2 if tag == "o" else 1),
                                      name=f"pjt{otp}n{nch}")
                    for kt in (4, 5):
                        nc.tensor.matmul(
                            pjn[:], wp_sb[:, kt, otp * 128:(otp + 1) * 128],
                            on_sb[kt][:, nch, :],
                            start=(kt == 4), stop=(kt == 5))
                    pj[otp] = pjn
                    proj_fin_nch(otp, nch, ap=pjn[:, :])

                # proj psums: otp0/1 -> "s" slots, otp2 -> "f"; a/b head
                # pipelines split DVE/Act so norms+copies run in parallel.
                # otp0/1 start at kt3 (prefill staged kt0-2), otp2-5 at kt4.
                proj_chunk(0, [3], 3, 5, tag="s")   # pads the ot-copy wait
                alo = new_o(ha, 0)
                o_chain(alo[0], ha, 0, 0)
                o_chain(alo[0], ha, 0, 1)
                o_chain(alo[0], ha, 0, 2)
                o_chain(alo[0], ha, 0, 3)
                o_norm(*alo)
                blo = new_o(hb, 0)
                o_chain(blo[0], hb, 0, 0)
                o_chain(blo[0], hb, 0, 1)
                # proj kt4 partials overlap the remaining O work (on4 ready)
                proj_chunk(1, [3], 3, 5, tag="s")
                o_chain(blo[0], hb, 0, 2)
                o_chain(blo[0], hb, 0, 3)
                o_norm(*blo, act=True)
                o_transpose(alo[1], ha, 0)
                proj_chunk(0, [4], 3, 5)
                o_transpose(blo[1], hb, 0, act=True)
                proj_chunk(1, [4], 3, 5)
                ahi = new_o(ha, 1)
                o_chain(ahi[0], ha, 1, 0)
                o_chain(ahi[0], ha, 1, 1)
                proj_chunk(2, [4], 4, 5, tag="f")
                # on5 nch0 complete: kt5 nch0 for otp0-2, fin nch0 eagerly
                proj_chunk(0, [5], 3, 5, nchs=(0,))
                proj_fin_nch(0, 0)
                o_chain(ahi[0], ha, 1, 2)
                o_chain(ahi[0], ha, 1, 3)
                o_norm(*ahi)
                proj_chunk(1, [5], 3, 5, nchs=(0,))
                proj_fin_nch(1, 0)
                bhi = new_o(hb, 1)
                o_chain(bhi[0], hb, 1, 0)
                o_chain(bhi[0], hb, 1, 1)
                proj_chunk(2, [5], 4, 5, nchs=(0,))
                proj_fin_nch(2, 0)
                o_chain(bhi[0], hb, 1, 2)
                o_chain(bhi[0], hb, 1, 3)
                o_norm(*bhi, act=True)
                o_transpose(ahi[1], ha, 1)
                o_transpose(bhi[1], hb, 1, act=True)
                # nch1 tails + remaining otps on the freed small slots
                proj_chunk(0, [5], 3, 5, nchs=(1,))
                proj_fin_nch(0, 1)
                proj_tail(3, 0, "o")
                proj_chunk(1, [5], 3, 5, nchs=(1,))
                proj_fin_nch(1, 1)
                proj_tail(3, 1, "o")
                proj_chunk(2, [5], 4, 5, nchs=(1,))
                proj_fin_nch(2, 1)
                proj_tail(4, 0, "o")
                proj_tail(4, 1, "o")
                proj_tail(5, 0, "f")
                proj_tail(5, 1, "f")

            if loop_r is not None:
                with tc.For_i(0, loop_r):
                    body()
            else:
                body()

    nc.compile()
    return nc


def _get_nc():
    if "nc" not in _CACHE:
        _CACHE["nc"] = _build_nc()
    return _CACHE["nc"]


def kernel(x, qkv_w, proj_w, proj_b):
    from concourse.bass_utils import run_bass_kernel_spmd

    nc = _get_nc()
    bf = ml_dtypes.bfloat16
    wqk = np.ascontiguousarray(
        qkv_w[:2 * C].T.reshape(CT, 128, 2, CT, 128)
        .transpose(2, 3, 1, 0, 4)).astype(bf)
    wv = np.ascontiguousarray(qkv_w[2 * C:].T).astype(bf).reshape(CT, 128, C)
    wp = np.ascontiguousarray(proj_w.T).astype(bf).reshape(CT, 128, C)
    pb = np.ascontiguousarray(proj_b.reshape(CT, 128).T).astype(np.float32)
    eye = np.eye(128, dtype=bf)
    in_maps = []
    for i in range(B):
        in_maps.append({
            "xT": np.ascontiguousarray(x[i].T).astype(bf),
            "wqk": wqk, "wv": wv, "wp": wp, "pb": pb, "eye": eye,
        })
    res = run_bass_kernel_spmd(nc, in_maps, core_ids=list(range(B)))
    out = np.stack([res.results[i]["out"].astype(np.float32).T for i in range(B)])
    return np.ascontiguousarray(out)


# revision 35
# speedup vs baseline: 1.0849x; 1.0382x over previous
"""Multi-head attention (B=8, N=1024, C=768, H=12) on 8 TRN2 NeuronCores.

Sharding: pure data-parallel over batch - core b computes attention for x[b].
Per-core Bass/Tile kernel, bf16 compute, f32 PSUM.

v3 schedule (orientation-B O):
  qkv/S unchanged from v2: qkv psum pairs heads on partition halves; S
  matmuls per (mt, nch) write [128 keys, 2 heads, 512 q] PSUM; one exp per
  (mt, nch) covers both heads -> E_ab[128, mt, nch, ab, 512] bf16.

  O restructured: out[q, d] = E_chunk^T @ v with E as the (free) stationary
  operand and v [128, 65] moving (ones col -> softmax sums in col 64).
  Per (head, qtile): 8 accumulating matmuls of 65 columns instead of the
  old [65, 512] orientation - halves the O column count on the PE.

  norm: DVE reciprocal of the PSUM sums column + per-partition
  tensor_scalar_mul -> normalized ob[q, d] bf16 in SBUF.

  transpose: proj needs on[d, q]; ob^T comes from a PE matmul against an
  identity matrix (ldweights are free): ot[64, 128] = ob_qt^T @ I, then one
  DVE copy per half moves [64, 4, 128] PSUM -> on_sb.

  PSUM budget (16KB/partition): tag "s" 2x[128,2,512]f32 (S double-buffer,
  also startup qk pair-0), tag "o" 2x[128,4,128]f32 (O accumulator halves
  and transpose outputs alternate through the same two slots), tag "f"
  1x4KB (warmup, qk/v/proj fillers, serial by construction).

  Per phase hp (S/exp of pair hp, O of pair hp-1), chains spread over mts:
    mt0 a-lo qt01 | mt1 a-lo qt23 +norm | mt2 b-lo qt01 | mt3 b-lo qt23
    +norm, T(a-lo)+copy | mt4 T(b-lo)+copy, a-hi qt45 | mt5 a-hi qt67
    +norm | mt6 b-hi qt45 | mt7 b-hi qt67 +norm, T(a-hi)+copy, T(b-hi)+copy
  so each "o" slot's next allocation waits only on work finished ~2 mts ago.

  tail: O(pair 5) same pattern dense; proj kt0-3 partials prestaged as
  phase-4/5 fillers (DVE stage to SBUF), tail does kt4-5 + fused epilogue
  (partial + psum + bias on DVE) + split-queue output DMA.
"""

import numpy as np
import ml_dtypes

B, N, C = 8, 1024, 768
H, D = 12, 64
SCALE = D ** -0.5
CT = C // 128        # 6 contraction tiles
NT = N // 128        # 8 token tiles
NCH = N // 512       # 2 n-chunks of 512
HP = H // 2          # 6 head pairs

_CACHE = {}


def _build_nc(loop_r=None):
    import concourse.bacc as bacc
    import concourse.mybir as mybir
    import concourse.tile as tile

    f32 = mybir.dt.float32
    bf16 = mybir.dt.bfloat16

    nc = bacc.Bacc("TRN2", target_bir_lowering=False, debug=False, num_devices=8)

    xT_d = nc.dram_tensor("xT", [C, N], bf16, kind="ExternalInput").ap()
    # weights declared row-tiled [CT, 128, ...] so one multi-descriptor DMA
    # (single HWDGE issue) can stage a whole weight with dst partitions =
    # the inner 128 rows
    # wqk layout [role, pair, row128, kt, d]: per-(role, pair-slice) DMA
    # opts to 3 dims (row, pair, kt*d) on both sides
    wqk_d = nc.dram_tensor("wqk", [2, CT, 128, CT, 128], bf16,
                           kind="ExternalInput").ap()
    wv_d = nc.dram_tensor("wv", [CT, 128, C], bf16, kind="ExternalInput").ap()
    wp_d = nc.dram_tensor("wp", [CT, 128, C], bf16, kind="ExternalInput").ap()
    pb_d = nc.dram_tensor("pb", [128, CT], f32, kind="ExternalInput").ap()
    eye_d = nc.dram_tensor("eye", [128, 128], bf16, kind="ExternalInput").ap()
    out_d = nc.dram_tensor("out", [C, N], bf16, kind="ExternalOutput").ap()

    with tile.TileContext(nc) as tc:
        with (
            tc.tile_pool(name="const", bufs=1) as cpool,
            tc.tile_pool(name="E", bufs=2) as epool,
            tc.tile_pool(name="qk", bufs=4) as qkpool,
            tc.tile_pool(name="small", bufs=4) as spool,
            tc.tile_pool(name="y", bufs=4) as ypool,
            tc.tile_pool(name="ps", bufs=1, space="PSUM") as pspool,
        ):
            # ---- persistent SBUF tensors ----
            xT_sb = cpool.tile([128, CT, N], bf16)            # 12KB/part
            wqk_sb = cpool.tile([128, 2, CT, C], bf16)        # 18KB
            wv_sb = cpool.tile([128, CT, C], bf16)            # 9KB
            wp_sb = cpool.tile([128, CT, C], bf16)            # 9KB
            pb_sb = cpool.tile([128, CT], f32)
            eye_sb = cpool.tile([128, 128], bf16)
            v_sb = [cpool.tile([128, H, D + 1], bf16, name=f"v{nt}")
                    for nt in range(NT)]                      # 12.2KB
            on_sb = [cpool.tile([128, NCH, 512], bf16, name=f"on{kt}")
                     for kt in range(CT)]                     # 12KB
            stg_sb = [cpool.tile([128, NCH, 512], bf16, name=f"stg{otp}")
                      for otp in range(CT)]                   # 12KB

            # warmup scratch memset first so PE ramp starts ASAP
            scr = cpool.tile([128, 256], bf16, name="scr")
            nc.vector.memset(scr[:], 1.0)

            # input DMA: everything HWDGE goes on the SYNC queue (SP has no
            # compute - issuing from scalar/vector blocks that engine's SEQ
            # behind the shared HWDGE device). Ordered by first use:
            # qk0 inputs, then pair-1 weights, wv, pairs 2-5, wp. xT rows
            # 1/3 ride the gpsimd SWDGE (separate device) in parallel.
            nc.sync.dma_start(xT_sb[:, 0, 0:512], xT_d[0:128, 0:512])
            nc.sync.dma_start(xT_sb[:, 0, 512:1024], xT_d[0:128, 512:1024])
            for role in range(2):
                nc.sync.dma_start(
                    wqk_sb[:, role, 0, :],
                    wqk_d[role, 0, :, :, :].rearrange("b c d -> b (c d)"))
            nc.gpsimd.dma_start(xT_sb[:, 1, :], xT_d[128:256, :])
            nc.gpsimd.dma_start(xT_sb[:, 3, :], xT_d[384:512, :])
            nc.sync.dma_start(xT_sb[:, 2, :], xT_d[256:384, :])
            nc.sync.dma_start(xT_sb[:, 4, :], xT_d[512:640, :])
            nc.sync.dma_start(xT_sb[:, 5, :], xT_d[640:768, :])
            for role in range(2):
                nc.sync.dma_start(
                    wqk_sb[:, role, 1, :],
                    wqk_d[role, 1, :, :, :].rearrange("b c d -> b (c d)"))
            nc.sync.dma_start(wv_sb[:, :, :],
                              wv_d.rearrange("a b c -> b a c"))
            for role in range(2):
                nc.sync.dma_start(
                    wqk_sb[:, role, 2:CT, :],
                    wqk_d[role, 2:CT, :, :, :]
                    .rearrange("a b c d -> b a (c d)"))
            nc.sync.dma_start(wp_sb[:, :, :],
                              wp_d.rearrange("a b c -> b a c"))
            nc.gpsimd.dma_start(pb_sb[:], pb_d[:])
            nc.gpsimd.dma_start(eye_sb[:], eye_d[:])
            # ones column fused into v (softmax sums emerge as O col 64)
            for nt in range(NT):
                nc.vector.memset(v_sb[nt][:, :, D:D + 1], 1.0)

            def body():
                qks = {}      # (hp, role) -> SBUF tile; ("ps",hp,role) -> psum
                vps = {}
                ps_w = pspool.tile([128, 2, 512], f32, tag="f", name="warm")
                for i in range(12):
                    nc.tensor.matmul(ps_w[:, 0, 0:256], scr[:, 0:128],
                                     scr[:], start=True, stop=True)

                def emit_qk_chunk(hp, role, lo, hi, tag="f",
                                  defer_copy=False):
                    key = ("ps", hp, role)
                    if lo == 0:
                        qks[key] = pspool.tile([128, NCH, 512], f32, tag=tag,
                                               bufs=(2 if tag == "s" else 1),
                                               name=f"qk{hp}r{role}")
                    ps = qks[key]
                    for kt in range(lo, hi):
                        for nch in range(NCH):
                            nc.tensor.matmul(
                                ps[:, nch, :],
                                wqk_sb[:, role, hp, kt * 128:kt * 128 + 128],
                                xT_sb[:, kt, nch * 512:(nch + 1) * 512],
                                start=(kt == 0), stop=(kt == CT - 1),
                            )
                    if hi == CT and not defer_copy:
                        qk_copy(hp, role)

                def qk_copy(hp, role):
                    # PSUM->SBUF copy emitted at a fixed low-DVE-pressure
                    # slot so it never delays a chain-gating norm/otcopy
                    key = ("ps", hp, role)
                    ps = qks[key]
                    t = qkpool.tile([128, NCH, 512], bf16, tag="qk",
                                    name=f"qk{hp}r{role}sb")
                    if hp == 0:
                        # startup: split copies across Act (idle) and DVE
                        for nch in range(NCH):
                            if role == 0:
                                nc.scalar.copy(t[:, nch, :], ps[:, nch, :])
                            else:
                                nc.vector.tensor_copy(t[:, nch, :],
                                                      ps[:, nch, :])
                    else:
                        nc.vector.tensor_copy(t[:], ps[:, :, :])
                    qks[(hp, role)] = t
                    del qks[key]

                def emit_v_chunk(nt, lo, hi):
                    # per-och 2KB psums ride the "o" slots (idle until the
                    # phase-1 O chains, which naturally wait on v's copies)
                    if lo == 0:
                        vps[nt] = [
                            pspool.tile([128, 8, 64], f32, tag="o", bufs=2,
                                        name=f"v{nt}o{och}")
                            for och in range(2)]
                    for kt in range(lo, hi):
                        for och in range(2):
                            nc.tensor.matmul(
                                vps[nt][och][:, 0:6, :],
                                xT_sb[:, kt, nt * 128:(nt + 1) * 128],
                                wv_sb[:, kt, och * 384:(och + 1) * 384],
                                start=(kt == 0), stop=(kt == CT - 1),
                            )
                    if hi == CT:
                        for och in range(2):
                            nc.vector.tensor_copy(
                                v_sb[nt][:, och * 6:(och + 1) * 6, 0:D],
                                vps[nt][och][:, 0:6, :],
                            )
                        del vps[nt]

                fillers = []

                def take_fillers(k):
                    for _ in range(min(k, len(fillers))):
                        fillers.pop(0)()

                def queue_qk_chunks(hp):
                    for role in (0, 1):
                        for kt in range(CT):
                            fillers.append(
                                lambda hp=hp, role=role, kt=kt:
                                emit_qk_chunk(hp, role, kt, kt + 1,
                                              defer_copy=(kt == CT - 1)))

                def queue_v_chunks(nt):
                    fillers.append(lambda: emit_v_chunk(nt, 0, 3))
                    fillers.append(lambda: emit_v_chunk(nt, 3, CT))

                pj = {}

                def stage_proj(otp):
                    nc.vector.tensor_copy(stg_sb[otp][:], pj[otp][:, :, :])
                    del pj[otp]

                def proj_chunk(otp, kts, lo, stop_kt, nchs=(0, 1), tag="f"):
                    if kts[0] == lo:
                        pj[otp] = pspool.tile([128, NCH, 512], f32, tag=tag,
                                              bufs=(2 if tag == "s" else 1),
                                              name=f"pj{otp}k{lo}")
                    for kt in kts:
                        for nch in nchs:
                            nc.tensor.matmul(
                                pj[otp][:, nch, :],
                                wp_sb[:, kt, otp * 128:(otp + 1) * 128],
                                on_sb[kt][:, nch, :],
                                start=(kt == lo), stop=(kt == stop_kt),
                            )

                def queue_proj_prefill(otps, last):
                    # prefill kt 0..last (on[last] must be ready a phase
                    # before the pops land), stage partial to SBUF
                    for otp in otps:
                        for kt in range(last + 1):
                            fillers.append(
                                lambda otp=otp, kt=kt:
                                proj_chunk(otp, [kt], 0, last))
                        fillers.append(lambda otp=otp: stage_proj(otp))

                # ---- startup: eager qk(pair 0) on the S psum slots;
                # v0/v1 go through the filler queue (wv lands late).
                # A warm matmul between chunks fills each DMA-wait gap so
                # the PE p-state ramp never resets ----
                for kt in range(CT):
                    emit_qk_chunk(0, 0, kt, kt + 1, tag="s")
                    emit_qk_chunk(0, 1, kt, kt + 1, tag="s")
                    nc.tensor.matmul(ps_w[:, 1, 0:256], scr[:, 0:128],
                                     scr[:], start=True, stop=True)
                # pair-1 role-0 fills the copy window before phase 0
                for kt in range(CT):
                    emit_qk_chunk(1, 0, kt, kt + 1)

                E_prev = None

                # ---- O machinery (orientation B) ----
                def o_chain(ps_o, h, half, qi, lo=0, hi=NT, E=None):
                    """ps_o[:, qi, 0:65] += E^T @ v over key subs lo..hi-1
                    (accumulation sub-order is free: start at 0, stop at 7)."""
                    nch = half
                    qc = qi
                    ab = h % 2
                    Esrc = E_prev if E is None else E
                    for sub in range(lo, hi):
                        nc.tensor.matmul(
                            ps_o[:, qi, 0:D + 1],
                            Esrc[:, sub, nch, ab, qc * 128:(qc + 1) * 128],
                            v_sb[sub][:, h, :],
                            start=(sub == 0), stop=(sub == NT - 1),
                        )

                def o_norm(ps_o, ob, rec, act=False):
                    """rec = 1/sums, ob[q, qt, d] = O * rec (bf16). act=True
                    runs the multiplies on the Activation engine (tail)."""
                    nc.vector.reciprocal_approx_fast(rec[:, :], ps_o[:, :, D])
                    for qi in range(4):
                        if act:
                            nc.scalar.activation(
                                ob[:, qi, :], ps_o[:, qi, 0:D],
                                mybir.ActivationFunctionType.Identity,
                                scale=rec[:, qi:qi + 1])
                        else:
                            nc.vector.tensor_scalar_mul(
                                ob[:, qi, :], ps_o[:, qi, 0:D],
                                rec[:, qi:qi + 1])

                def o_transpose(ob, h, half, act=False):
                    """ot = ob^T per qtile via eye matmul; copy to on_sb."""
                    hp0, odd = h // 2, h % 2
                    ot = pspool.tile([64, 4, 128], f32, tag="o",
                                     bufs=2, name=f"ot{h}h{half}")
                    for qi in range(4):
                        nc.tensor.matmul(ot[0:64, qi, :], ob[:, qi, :],
                                         eye_sb[:], start=True, stop=True)
                    dst = on_sb[hp0][64 * odd:64 * odd + 64, half, :]
                    if act:
                        nc.scalar.copy(dst, ot[0:64, :, :])
                    else:
                        nc.vector.tensor_copy(dst, ot[0:64, :, :])

                def new_o(h, half):
                    ps_o = pspool.tile([128, 4, 128], f32, tag="o",
                                       bufs=2, name=f"o{h}h{half}")
                    ob = ypool.tile([128, 4, D], bf16, tag="ob", bufs=4,
                                    name=f"ob{h}h{half}")
                    rec = spool.tile([128, 4], f32, tag="rec",
                                     name=f"rec{h}h{half}")
                    return ps_o, ob, rec

                for hp in range(HP):
                    q_sb, k_sb = qks[(hp, 0)], qks[(hp, 1)]
                    E_ab = epool.tile([128, NT, NCH, 2, 512], bf16, tag="E",
                                      name=f"E{hp}")
                    if hp == 0:
                        for kt in range(CT):
                            fillers.append(
                                lambda kt=kt:
                                emit_qk_chunk(1, 1, kt, kt + 1,
                                              defer_copy=(kt == CT - 1)))
                        for nt in range(NT):
                            queue_v_chunks(nt)
                    elif hp < HP - 1:
                        queue_qk_chunks(hp + 1)
                        if hp == HP - 2:
                            queue_proj_prefill((0, 1), 2)
                    else:
                        queue_proj_prefill((2, 3, 4, 5), 3)

                    ha, hb = (2 * (hp - 1), 2 * (hp - 1) + 1)
                    st = {}
                    for mt in range(NT):
                        for nch in range(NCH):
                            ps_s = pspool.tile([128, 2, 512], f32, tag="s",
                                               bufs=2, name=f"s{hp}m{mt}n{nch}")
                            c0, c1 = mt // 4, (mt % 4) * 128
                            nc.tensor.matmul(
                                ps_s[:, 0, :],
                                k_sb[0:64, c0, c1:c1 + 128],
                                q_sb[0:64, nch, :], start=True, stop=True,
                            )
                            nc.tensor.matmul(
                                ps_s[:, 1, :],
                                k_sb[64:128, c0, c1:c1 + 128],
                                q_sb[64:128, nch, :], start=True, stop=True,
                            )
                            nc.scalar.activation(
                                E_ab[:, mt, nch, :, :], ps_s[:, :, :],
                                mybir.ActivationFunctionType.Exp, scale=SCALE,
                            )
                        if mt == 2 and 0 < hp < HP - 1:
                            qk_copy(hp + 1, 0)
                        if mt == 6 and hp < HP - 1:
                            qk_copy(hp + 1, 1)
                        if E_prev is not None and hp < HP - 1:
                            if mt == 0:
                                st["alo"] = new_o(ha, 0)
                                o_chain(st["alo"][0], ha, 0, 0)
                                o_chain(st["alo"][0], ha, 0, 1)
                            elif mt == 1:
                                o_chain(st["alo"][0], ha, 0, 2)
                                o_chain(st["alo"][0], ha, 0, 3)
                                o_norm(*st["alo"])
                            elif mt == 2:
                                st["blo"] = new_o(hb, 0)
                                o_chain(st["blo"][0], hb, 0, 0)
                                o_chain(st["blo"][0], hb, 0, 1)
                            elif mt == 3:
                                o_chain(st["blo"][0], hb, 0, 2)
                                o_chain(st["blo"][0], hb, 0, 3)
                                o_norm(*st["blo"])
                                o_transpose(st["alo"][1], ha, 0)
                            elif mt == 4:
                                o_transpose(st["blo"][1], hb, 0)
                                st["ahi"] = new_o(ha, 1)
                                o_chain(st["ahi"][0], ha, 1, 0)
                                o_chain(st["ahi"][0], ha, 1, 1)
                            elif mt == 5:
                                o_chain(st["ahi"][0], ha, 1, 2)
                                o_chain(st["ahi"][0], ha, 1, 3)
                                o_norm(*st["ahi"])
                            elif mt == 6:
                                st["bhi"] = new_o(hb, 1)
                                o_chain(st["bhi"][0], hb, 1, 0)
                                o_chain(st["bhi"][0], hb, 1, 1)
                            elif mt == 7:
                                o_chain(st["bhi"][0], hb, 1, 2)
                                o_chain(st["bhi"][0], hb, 1, 3)
                                o_norm(*st["bhi"])
                        elif E_prev is not None:
                            # phase 5: pair-4 O compressed to mts 0-5, then
                            # the tail heads' accumulations pre-start during
                            # mts 6-7 as this phase's exps land
                            if mt == 0:
                                st["alo"] = new_o(ha, 0)
                                for qi in range(4):
                                    o_chain(st["alo"][0], ha, 0, qi)
                            elif mt == 1:
                                o_norm(*st["alo"])
                                st["blo"] = new_o(hb, 0)
                                for qi in range(4):
                                    o_chain(st["blo"][0], hb, 0, qi)
                            elif mt == 2:
                                o_norm(*st["blo"], act=True)
                                o_transpose(st["alo"][1], ha, 0)
                            elif mt == 3:
                                o_transpose(st["blo"][1], hb, 0, act=True)
                                st["ahi"] = new_o(ha, 1)
                                for qi in range(4):
                                    o_chain(st["ahi"][0], ha, 1, qi)
                            elif mt == 4:
                                o_norm(*st["ahi"])
                                st["bhi"] = new_o(hb, 1)
                                for qi in range(4):
                                    o_chain(st["bhi"][0], hb, 1, qi)
                            elif mt == 5:
                                o_transpose(st["ahi"][1], ha, 1)
                                o_norm(*st["bhi"], act=True)
                            elif mt == 6:
                                o_transpose(st["bhi"][1], hb, 1, act=True)
                                st["t10"] = new_o(H - 2, 0)
                            elif mt == 7:
                                st["t11"] = new_o(H - 1, 0)
                        nfill = 3 if (mt < 2 or len(fillers) > 8) else 2
                        take_fillers(nfill)
                    if E_prev is not None and hp < HP - 1:
                        o_transpose(st["ahi"][1], ha, 1)
                        take_fillers(2)
                        o_transpose(st["bhi"][1], hb, 1)
                    take_fillers(len(fillers))
                    E_prev = E_ab
                    tst = st

                # ---- tail: O(pair 5) + proj kt4-5 + epilogues ----
                ha, hb = H - 2, H - 1

                yts = {}

                def proj_fin_nch(otp, nch, ap=None, dve=False):
                    # two fin flavors, alternated to balance the tail:
                    # Act(psum+bias) + DVE tt-add, or a single DVE stt
                    if otp not in yts:
                        yts[otp] = (
                            ypool.tile([128, NCH, 512], bf16, tag="y1",
                                       name=f"y1_{otp}"),
                            ypool.tile([128, NCH, 512], bf16, tag="yt",
                                       name=f"yt_{otp}"),
                        )
                    y1, yt = yts[otp]
                    src_ap = pj[otp][:, nch, :] if ap is None else ap
                    if dve:
                        nc.vector.scalar_tensor_tensor(
                            yt[:, nch, :], src_ap, pb_sb[:, otp:otp + 1],
                            stg_sb[otp][:, nch, :],
                            op0=mybir.AluOpType.add,
                            op1=mybir.AluOpType.add,
                        )
                    else:
                        nc.scalar.activation(
                            y1[:, nch, :], src_ap,
                            mybir.ActivationFunctionType.Identity,
                            bias=pb_sb[:, otp:otp + 1],
                        )
                        nc.vector.tensor_tensor(
                            yt[:, nch, :], y1[:, nch, :],
                            stg_sb[otp][:, nch, :],
                            op=mybir.AluOpType.add,
                        )
                    eng = nc.gpsimd if nch == 0 else nc.sync
                    eng.dma_start(
                        out_d[otp * 128:(otp + 1) * 128,
                              nch * 512:(nch + 1) * 512],
                        yt[:, nch, :])

                def proj_fin_staged(otp):
                    for nch in range(NCH):
                        proj_fin_nch(otp, nch)

                def proj_tail(otp, nch, tag):
                    # wave-2 proj: kt4-5 straight into a freed small slot,
                    # fin immediately (Act + DVE + DMA)
                    pjn = pspool.tile([128, 512], f32, tag=tag,
                                      bufs=(2 if tag == "o" else 1),
                                      name=f"pjt{otp}n{nch}")
                    for kt in (4, 5):
                        nc.tensor.matmul(
                            pjn[:], wp_sb[:, kt, otp * 128:(otp + 1) * 128],
                            on_sb[kt][:, nch, :],
                            start=(kt == 4), stop=(kt == 5))
                    pj[otp] = pjn
                    proj_fin_nch(otp, nch, ap=pjn[:, :], dve=True)

                # proj psums: otp0/1 -> "s" slots, otp2 -> "f"; wave-2
                # otps on the freed 2KB "o"/"f" slots. t10/t11 lo-halves were
                # pre-accumulated (subs 0-6) during phase-5 mts 6-7.
                t10, t11 = tst["t10"], tst["t11"]
                # both hi-half accumulators share one 4KB "f" tile so their
                # chains run immediately, parallel to the lo-half norm path
                thi = pspool.tile([128, 8, 128], f32, tag="f", name="thi")
                ob10h = ypool.tile([128, 4, D], bf16, tag="ob", bufs=4,
                                   name="ob10h")
                ob11h = ypool.tile([128, 4, D], bf16, tag="ob", bufs=4,
                                   name="ob11h")
                rec10h = spool.tile([128, 4], f32, tag="rec", name="rec10h")
                rec11h = spool.tile([128, 4], f32, tag="rec", name="rec11h")
                for qi in range(4):
                    o_chain(t10[0], ha, 0, qi, 0, 8, E=E_prev)
                for qi in range(4):
                    o_chain(t11[0], hb, 0, qi, 0, 8, E=E_prev)
                for qi in range(4):
                    o_chain(thi[:, 0:4, :], ha, 1, qi, E=E_prev)
                o_norm(*t10)
                o_norm(*t11, act=True)
                for qi in range(4):
                    o_chain(thi[:, 4:8, :], hb, 1, qi, E=E_prev)
                o_transpose(t10[1], ha, 0)
                proj_chunk(0, [3], 3, 5, tag="s")
                o_transpose(t11[1], hb, 0, act=True)
                o_norm(thi[:, 0:4, :], ob10h, rec10h)
                o_norm(thi[:, 4:8, :], ob11h, rec11h, act=True)
                proj_chunk(1, [3], 3, 5, tag="s")
                o_transpose(ob10h, ha, 1)
                proj_chunk(0, [4], 3, 5)
                o_transpose(ob11h, hb, 1, act=True)
                proj_chunk(1, [4], 3, 5)
                proj_chunk(2, [4], 4, 5, tag="f")
                # on5 nch0 complete: kt5 nch0 for otp0-2, fin eagerly
                proj_chunk(0, [5], 3, 5, nchs=(0,))
                proj_fin_nch(0, 0)
                proj_chunk(1, [5], 3, 5, nchs=(0,))
                proj_fin_nch(1, 0)
                proj_chunk(2, [5], 4, 5, nchs=(0,))
                proj_fin_nch(2, 0)
                proj_tail(3, 0, "o")
                # wave-2: otp3 on the "o" slots, otp4/5 on the freed "s"
                # slots (allocated only after pj0/pj1 fully drain them)
                proj_chunk(0, [5], 3, 5, nchs=(1,))
                proj_fin_nch(0, 1)
                proj_chunk(4, [4], 4, 5, tag="s")
                proj_tail(3, 1, "o")
                proj_chunk(1, [5], 3, 5, nchs=(1,))
                proj_fin_nch(1, 1)
                proj_chunk(5, [4], 4, 5, tag="s")
                proj_chunk(4, [5], 4, 5, nchs=(0,))
                proj_fin_nch(4, 0)
                proj_chunk(2, [5], 4, 5, nchs=(1,))
                proj_fin_nch(2, 1)
                proj_chunk(5, [5], 4, 5, nchs=(0,))
                proj_fin_nch(5, 0)
                proj_chunk(4, [5], 4, 5, nchs=(1,))
                proj_fin_nch(4, 1)
                proj_chunk(5, [5], 4, 5, nchs=(1,))
                proj_fin_nch(5, 1, dve=True)

            if loop_r is not None:
                with tc.For_i(0, loop_r):
                    body()
            else:
                body()

    nc.compile()
    return nc


def _get_nc():
    if "nc" not in _CACHE:
        _CACHE["nc"] = _build_nc()
    return _CACHE["nc"]


def kernel(x, qkv_w, proj_w, proj_b):
    from concourse.bass_utils import run_bass_kernel_spmd

    nc = _get_nc()
    bf = ml_dtypes.bfloat16
    wqk = np.ascontiguousarray(
        qkv_w[:2 * C].T.reshape(CT, 128, 2, CT, 128)
        .transpose(2, 3, 1, 0, 4)).astype(bf)
    wv = np.ascontiguousarray(qkv_w[2 * C:].T).astype(bf).reshape(CT, 128, C)
    wp = np.ascontiguousarray(proj_w.T).astype(bf).reshape(CT, 128, C)
    pb = np.ascontiguousarray(proj_b.reshape(CT, 128).T).astype(np.float32)
    eye = np.eye(128, dtype=bf)
    in_maps = []
    for i in range(B):
        in_maps.append({
            "xT": np.ascontiguousarray(x[i].T).astype(bf),
            "wqk": wqk, "wv": wv, "wp": wp, "pb": pb, "eye": eye,
        })
    res = run_bass_kernel_spmd(nc, in_maps, core_ids=list(range(B)))
    out = np.stack([res.results[i]["out"].astype(np.float32).T for i in range(B)])
    return np.ascontiguousarray(out)
